# revision 23
# baseline (speedup 1.0000x reference)
"""Causal self-attention (B=2, T=2048, C=1024, H=16) on 8 TRN2 NeuronCores.

Sharding: tensor-parallel over heads (2 heads/core) for QKV projection and
attention; AllToAll converts the head-sharded attention output into a
sequence-sharded layout; each core then computes its 512-row slice of the
output projection. Host only slices/casts inputs and concatenates outputs.

Device math in bf16 with fp32 PSUM accumulation:
  - x is pre-transposed on host to xT [C, B*T] (bf16) so every matmul
    contraction has channels on the partition axis.
  - Scores are built transposed (S^T [keys, queries]); both heads' score
    matmuls are row-tiled (K=64 halves of the PE array) and run
    concurrently; both land in one 2-bank PSUM tile so a single exp
    covers both heads.
  - PV matmuls are col-tiled (M=64 halves) and run concurrently.
  - Softmax sums: P tiles are accumulated on DVE; one col-tiled
    broadcast-matmul pair per chunk produces [128,512] broadcast sums,
    so normalization is two full-width DVE ops (reciprocal + mul).
  - Causal slicing: diagonal key-tiles only compute/exp the valid
    query-column range; a single [128,2,128] tril slab handles masking.
  - Output projection packs two 64-row query strips into M=128 matmuls;
    biases are added by fused DVE adds (no bias matmuls).
"""
import os
import math
import threading

import numpy as np
import ml_dtypes

import concourse.bass as bass
import concourse.tile as tile
from concourse import mybir, bacc, bass_utils

B, T, C, H = 2, 2048, 1024, 16
D = C // H                 # 64
NCORES = 8
HPC = H // NCORES          # heads per core = 2
HC = HPC * D               # head-channels per core = 128
BT = B * T                 # 4096
TQ = 512                   # query chunk
TKT = 128                  # key tile
ROWS = BT // NCORES        # output rows per core = 512
SM_SCALE = 1.0 / math.sqrt(D)
KT = C // 128              # 8 contraction tiles over channels
NCH = BT // TQ             # 8 T-chunks over B*T
SPC = TQ // D              # 8 strips of 64 rows per chunk (one per core)

F32 = mybir.dt.float32
BF16 = mybir.dt.bfloat16
BF16_NP = ml_dtypes.bfloat16


def _build_program():
    nc = bacc.Bacc("TRN2", target_bir_lowering=False, debug=False,
                   num_devices=NCORES)
    xt = nc.dram_tensor("xt", [C, BT], BF16, kind="ExternalInput").ap()
    wqkv = nc.dram_tensor("wqkv", [C, 3 * HC], BF16, kind="ExternalInput").ap()
    wproj = nc.dram_tensor("wproj", [C, C], BF16, kind="ExternalInput").ap()
    bqk = nc.dram_tensor("bqk", [HC, 2], F32, kind="ExternalInput").ap()
    bvb = nc.dram_tensor("bvb", [128, HC], BF16, kind="ExternalInput").ap()
    bpb = nc.dram_tensor("bpb", [128, C], BF16, kind="ExternalInput").ap()
    mask2 = nc.dram_tensor("mask2", [TKT, 2, TKT], BF16,
                           kind="ExternalInput").ap()
    outp = nc.dram_tensor("out", [ROWS, C], F32, kind="ExternalOutput").ap()

    with tile.TileContext(nc) as tc:
        with (
            tc.tile_pool(name="consts", bufs=1) as consts,
            tc.tile_pool(name="xpool", bufs=2) as xpool,
            tc.tile_pool(name="ppool", bufs=6) as ppool,
            tc.tile_pool(name="apool", bufs=3) as apool,
            tc.tile_pool(name="npool", bufs=2) as npool,
            tc.tile_pool(name="opool", bufs=2) as opool,
            tc.tile_pool(name="ps_ss", bufs=2, space="PSUM") as ps_ss,
            tc.tile_pool(name="ps_y", bufs=2, space="PSUM") as ps_y,
            tc.tile_pool(name="ps_sum", bufs=1, space="PSUM") as ps_sum,
            tc.tile_pool(name="ps_o", bufs=1, space="PSUM") as ps_o,
            tc.tile_pool(name="dram", bufs=1, space="DRAM") as dram,
        ):
            # ---- stage 0: weights & constants (single-trigger batched
            # DMAs: the ~600ns per-dma_start engine cost dominates startup
            # otherwise) ----
            bqk_sb = consts.tile([HC, 2], F32, name="bqk_sb")
            nc.sync.dma_start(out=bqk_sb, in_=bqk)
            bq_sb = bqk_sb[:, 0:1]
            bk_sb = bqk_sb[:, 1:2]
            # one medium DMA per section/kt rather than one huge transfer:
            # each trigger's packets land on one HW queue (~83 GB/s), so
            # several in-flight triggers parallelize across queues
            wqkv_sb = consts.tile([128, KT, 3 * HC], BF16, name="wqkv_sb")
            for sec in range(3):
                nc.sync.dma_start(
                    out=wqkv_sb[:, :, HC * sec:HC * (sec + 1)],
                    in_=wqkv[:, HC * sec:HC * (sec + 1)]
                        .rearrange("(k p) c -> p k c", p=128))
            # prefetch all xt chunk-pairs on the two hardware-DGE engines
            # (sync + scalar); gpsimd is software-DGE with slow, variable
            # descriptor generation — late-needed weights only
            xt_pair = []
            for p in range(4):
                x1 = xpool.tile([128, KT, 2 * TQ], BF16, tag="xt",
                                name=f"xt_{p}")
                for kt in range(KT):
                    if p == 0:
                        eng = nc.sync if kt % 2 == 0 else nc.scalar
                    else:
                        eng = nc.sync if p == 2 else nc.scalar
                    eng.dma_start(
                        out=x1[:, kt, :],
                        in_=xt[128 * kt:128 * (kt + 1),
                               1024 * p:1024 * p + 1024])
                xt_pair.append(x1)
            bvb_sb = consts.tile([128, HC], BF16, name="bvb_sb")
            nc.gpsimd.dma_start(out=bvb_sb, in_=bvb)
            # big weights not needed until late: after the xt prefetches
            wproj_sb = consts.tile([128, KT, C], BF16, name="wproj_sb")
            nc.gpsimd.dma_start(out=wproj_sb,
                                in_=wproj.rearrange("(k p) c -> p k c", p=128))
            bpb_sb = consts.tile([128, C], BF16, name="bpb_sb")
            nc.gpsimd.dma_start(out=bpb_sb, in_=bpb)
            mask_sb = consts.tile([TKT, 2, TKT], BF16, name="mask_sb")
            nc.gpsimd.dma_start(out=mask_sb, in_=mask2)
            ones64 = consts.tile([128, D], BF16, name="ones64")
            nc.vector.memset(ones64, 1.0)

            qT_b = [consts.tile([HC, T], BF16, name=f"qT_sb{b}")
                    for b in range(B)]
            kT_b = [consts.tile([HC, T], BF16, name=f"kT_sb{b}")
                    for b in range(B)]
            # v tiles: [128 keys, 128] = [v_h0 (64) | v_h1 (64)]
            v_sb = [consts.tile([128, HC], BF16, name=f"v_sb{tt}")
                    for tt in range(BT // 128)]

            # per-pair exchange buffers: block s holds both chunks' strips
            # (queries [64s, 64s+64) of each chunk) for core s
            a2a_in = [dram.tile([NCORES, 2, HC, D], BF16, name=f"a2a_in{p}")
                      for p in range(4)]
            a2a_out = [dram.tile([NCORES, 2, HC, D], BF16, name=f"a2a_out{p}")
                       for p in range(4)]

            # ---- stage 1: QKV projection (both batches up front) ----
            for b in range(B):
                for cp in range(2):                      # chunk pairs
                    xx = xt_pair[2 * b + cp]
                    for half in range(2):
                        cl = 2 * cp + half               # chunk in batch
                        c = 4 * b + cl                   # global chunk
                        col = slice(TQ * half, TQ * (half + 1))
                        ps = ps_ss.tile([128, 2 * TQ], F32, tag="ss")
                        for kt in range(KT):
                            nc.tensor.matmul(
                                ps[:, 0:TQ],
                                lhsT=wqkv_sb[:, kt, 0:HC],
                                rhs=xx[:, kt, col],
                                start=(kt == 0), stop=(kt == KT - 1))
                        for kt in range(KT):
                            nc.tensor.matmul(
                                ps[:, TQ:2 * TQ],
                                lhsT=wqkv_sb[:, kt, HC:2 * HC],
                                rhs=xx[:, kt, col],
                                start=(kt == 0), stop=(kt == KT - 1))
                        nc.scalar.activation(
                            out=qT_b[b][:, TQ * cl:TQ * (cl + 1)],
                            in_=ps[:, 0:TQ],
                            func=mybir.ActivationFunctionType.Identity,
                            bias=bq_sb, scale=1.0)
                        nc.scalar.activation(
                            out=kT_b[b][:, TQ * cl:TQ * (cl + 1)],
                            in_=ps[:, TQ:2 * TQ],
                            func=mybir.ActivationFunctionType.Identity,
                            bias=bk_sb, scale=SM_SCALE)
                        for s in range(TQ // 128):
                            tt = 4 * c + s
                            pv = ps_sum.tile([128, HC], F32, tag="sum")
                            for kt in range(KT):
                                nc.tensor.matmul(
                                    pv,
                                    lhsT=xx[:, kt, TQ * half + 128 * s:
                                            TQ * half + 128 * (s + 1)],
                                    rhs=wqkv_sb[:, kt, 2 * HC:3 * HC],
                                    start=(kt == 0), stop=(kt == KT - 1))
                            nc.vector.tensor_add(v_sb[tt], pv, bvb_sb)

            # ---- stage 2: attention; two chunks in flight, biggest first;
            # each pair's exchange + output projection follows immediately
            # and hides under the next pair's attention ----
            def emit_S(b, jl, slot):
                """Issue both heads' score matmuls + exp (+ mask) for key
                tile `slot` of chunk jl; returns (pt tile, c0)."""
                i = slot
                q0 = TQ * jl
                k0 = TKT * i
                r = i - (TQ // TKT) * jl
                c0 = TKT * r if r >= 0 else 0
                ss = ps_ss.tile([128, 2 * TQ], F32, tag="ss")
                nc.tensor.matmul(
                    ss[:, c0:TQ],
                    lhsT=kT_b[b][0:D, k0:k0 + TKT],
                    rhs=qT_b[b][0:D, q0 + c0:q0 + TQ],
                    start=True, stop=True)
                nc.tensor.matmul(
                    ss[:, TQ + c0:2 * TQ],
                    lhsT=kT_b[b][D:2 * D, k0:k0 + TKT],
                    rhs=qT_b[b][D:2 * D, q0 + c0:q0 + TQ],
                    start=True, stop=True)
                pt = ppool.tile([128, 2, TQ], BF16, tag="pt")
                ssv = ss.rearrange("p (h q) -> p h q", h=2)
                nc.scalar.activation(
                    out=pt[:, :, c0:TQ], in_=ssv[:, :, c0:TQ],
                    func=mybir.ActivationFunctionType.Exp)
                if r >= 0:
                    nc.vector.tensor_mul(pt[:, :, c0:c0 + TKT],
                                         pt[:, :, c0:c0 + TKT], mask_sb)
                return pt, c0

            def emit_PV(b, jl, i, nkt, y_ps, acc, pt, c0):
                vt = v_sb[(T // 128) * b + i]
                ptf = pt.rearrange("p h q -> p (h q)")
                nc.tensor.matmul(
                    y_ps[0:D, c0:TQ],
                    lhsT=vt[:, 0:D],
                    rhs=ptf[:, c0:TQ],
                    start=(i == 0), stop=(i == nkt - 1),
                    skip_group_check=True)
                nc.tensor.matmul(
                    y_ps[D:2 * D, c0:TQ],
                    lhsT=vt[:, D:2 * D],
                    rhs=ptf[:, TQ + c0:2 * TQ],
                    start=(i == 0), stop=(i == nkt - 1),
                    skip_group_check=True)
                if i == 0:
                    nc.vector.tensor_copy(out=acc, in_=ptf)
                else:
                    accv = acc.rearrange("p (h q) -> p h q", h=2)
                    nc.vector.tensor_add(accv[:, :, c0:TQ],
                                         accv[:, :, c0:TQ],
                                         pt[:, :, c0:TQ])

            def finalize(pi, which, y_ps, acc):
                sums = ps_sum.tile([128, TQ], F32, tag="sum")
                nc.tensor.matmul(sums[0:D, :], lhsT=ones64,
                                 rhs=acc[:, 0:TQ], start=True, stop=True)
                nc.tensor.matmul(sums[D:2 * D, :], lhsT=ones64,
                                 rhs=acc[:, TQ:2 * TQ], start=True, stop=True)
                recip = npool.tile([128, TQ], F32, tag="recip")
                nc.vector.reciprocal_approx_fast(out=recip, in_=sums)
                yt = npool.tile([128, TQ], BF16, tag="yt")
                nc.vector.tensor_mul(yt, y_ps, recip)
                nc.gpsimd.dma_start(
                    out=a2a_in[pi][:, which].rearrange("s p q -> p s q"),
                    in_=yt.rearrange("p (s q) -> p s q", s=SPC))

            def stage4_pair(pi, cA, cB):
                """Output projection for two 64-row strips, M=128-packed."""
                yy = opool.tile([128, KT, 2 * D], BF16, tag="ylhs")
                nc.gpsimd.dma_start(
                    out=yy.rearrange("p k (w q) -> p k w q", w=2),
                    in_=a2a_out[pi].rearrange("k w p q -> p k w q"))
                osb = opool.tile([128, 2 * TQ], F32, tag="osb")
                for n in range(C // TQ):
                    po = ps_o.tile([128, TQ], F32, tag="po")
                    for kt in range(KT):
                        nc.tensor.matmul(
                            po,
                            lhsT=yy[:, kt, :],
                            rhs=wproj_sb[:, kt, TQ * n:TQ * (n + 1)],
                            start=(kt == 0), stop=(kt == KT - 1))
                    nc.vector.tensor_add(osb[:, TQ * n:TQ * (n + 1)], po,
                                         bpb_sb[:, TQ * n:TQ * (n + 1)])
                for ci, cc in enumerate((cA, cB)):
                    nc.sync.dma_start(
                        out=outp[D * cc:D * (cc + 1), :],
                        in_=osb[D * ci:D * (ci + 1), :])

            pending_s4 = []
            pi = -1
            for b in range(B):
                for jA, jB in ((1, 0), (3, 2)):
                    pi += 1
                    cA, cB = 4 * b + jA, 4 * b + jB
                    nktA, nktB = 4 * (jA + 1), 4 * (jB + 1)
                    yA = ps_y.tile([128, TQ], F32, tag="y", name="yA")
                    yB = ps_y.tile([128, TQ], F32, tag="y", name="yB")
                    accA = apool.tile([128, 2 * TQ], BF16, tag="acc",
                                      name="accA")
                    accB = apool.tile([128, 2 * TQ], BF16, tag="acc",
                                      name="accB")
                    prevA = prevB = None
                    for i in range(nktA):
                        curA = emit_S(b, jA, i)
                        if prevA is not None:
                            emit_PV(b, jA, i - 1, nktA, yA, accA, *prevA)
                        prevA = curA
                        if i < nktB:
                            curB = emit_S(b, jB, i)
                            if prevB is not None:
                                emit_PV(b, jB, i - 1, nktB, yB, accB, *prevB)
                            prevB = curB
                        elif i == nktB:
                            # B is done: flush its last PV and stage its half
                            # of the pair's exchange buffer immediately
                            emit_PV(b, jB, nktB - 1, nktB, yB, accB, *prevB)
                            finalize(pi, 1, yB, accB)
                        if pi == 3 and i in (2, 6, 10) and pending_s4:
                            # all earlier pairs' output projections fire
                            # inside the LAST pair's attention: their a2a
                            # exchanges are long done by now, so the matmuls
                            # never head-of-line block the PE queue, and
                            # attention in pairs 0-2 runs stall-free
                            pending_s4.pop(0)()
                    emit_PV(b, jA, nktA - 1, nktA, yA, accA, *prevA)
                    finalize(pi, 0, yA, accA)
                    nc.gpsimd.collective_compute(
                        "AllToAll", mybir.AluOpType.bypass,
                        replica_groups=[list(range(NCORES))],
                        ins=[a2a_in[pi].opt()],
                        outs=[a2a_out[pi].opt()])
                    pending_s4.append(lambda pi=pi, cA=cA, cB=cB:
                                      stage4_pair(pi, cA, cB))
            for s4 in pending_s4:
                s4()

    nc.compile()
    return nc


_lock = threading.Lock()
_cached_nc = None
last_results = None  # BassKernelResults of the most recent kernel() call


def _get_program():
    global _cached_nc
    with _lock:
        if _cached_nc is None:
            _cached_nc = _build_program()
    return _cached_nc


def _host_inputs(x, W_qkv, b_qkv, W_proj, b_proj):
    bf = lambda a: np.ascontiguousarray(a).astype(BF16_NP)
    x = np.asarray(x, dtype=np.float32)
    W_qkv = np.asarray(W_qkv, dtype=np.float32)
    b_qkv = np.asarray(b_qkv, dtype=np.float32)
    W_proj = np.asarray(W_proj, dtype=np.float32)
    b_proj = np.asarray(b_proj, dtype=np.float32)

    xt = bf(x.reshape(BT, C).T)                     # [C, BT]
    wproj = bf(W_proj)                              # [C, C]
    bpb = bf(np.broadcast_to(b_proj.reshape(1, C), (128, C)))
    k = np.arange(TKT)[:, None, None]
    q = np.arange(TKT)[None, None, :]
    mask2 = np.broadcast_to(k <= q, (TKT, 2, TKT)).astype(BF16_NP)

    in_maps = []
    for i in range(NCORES):
        sel = slice(HC * i, HC * (i + 1))
        wq = W_qkv[:, sel]
        wk = W_qkv[:, C + HC * i:C + HC * (i + 1)]
        wv = W_qkv[:, 2 * C + HC * i:2 * C + HC * (i + 1)]
        bv = b_qkv[2 * C + HC * i:2 * C + HC * (i + 1)]
        in_maps.append({
            "xt": xt,
            "wqkv": bf(np.concatenate([wq, wk, wv], axis=1)),
            "wproj": wproj,
            "bqk": np.ascontiguousarray(np.stack(
                [b_qkv[sel],
                 b_qkv[C + HC * i:C + HC * (i + 1)] * SM_SCALE],
                axis=1)).astype(np.float32),
            "bvb": bf(np.broadcast_to(bv.reshape(1, HC), (128, HC))),
            "bpb": bpb,
            "mask2": np.ascontiguousarray(mask2),
        })
    return in_maps


def kernel(x, W_qkv, b_qkv, W_proj, b_proj):
    global last_results
    nc = _get_program()
    in_maps = _host_inputs(x, W_qkv, b_qkv, W_proj, b_proj)
    trace = bool(int(os.environ.get("KERNEL_TRACE", "0")))
    res = bass_utils.run_bass_kernel_spmd(
        nc, in_maps, core_ids=list(range(NCORES)), trace=trace)
    last_results = res
    # core s's output rows are strip s (64 rows) of every 512-row chunk
    arr = np.stack([res.results[s]["out"].reshape(BT // TQ, D, C)
                    for s in range(NCORES)], axis=1)   # [chunk, core, 64, C]
    return np.ascontiguousarray(arr.reshape(B, T, C))


# revision 26
# speedup vs baseline: 1.2292x; 1.2292x over previous
"""Causal self-attention (B=2, T=2048, C=1024, H=16) on 8 TRN2 NeuronCores.

Sharding: tensor-parallel over heads (2 heads/core) for QKV projection and
attention; AllToAll converts the head-sharded attention output into a
sequence-sharded layout; each core then computes its 512-row slice of the
output projection. Host only slices/casts inputs and concatenates outputs.

Device math in bf16 with fp32 PSUM accumulation:
  - x is pre-transposed on host to xT [C, B*T] (bf16) so every matmul
    contraction has channels on the partition axis.
  - Scores are built transposed (S^T [keys, queries]); both heads' score
    matmuls are row-tiled (K=64 halves of the PE array) and run
    concurrently; both land in one 2-bank PSUM tile so a single exp
    covers both heads.
  - PV matmuls are col-tiled (M=64 halves) and run concurrently.
  - Softmax sums: P tiles are accumulated on DVE; one col-tiled
    broadcast-matmul pair per chunk produces [128,512] broadcast sums,
    so normalization is two full-width DVE ops (reciprocal + mul).
  - Causal slicing: diagonal key-tiles only compute/exp the valid
    query-column range; a single [128,2,128] tril slab handles masking.
  - Output projection packs two 64-row query strips into M=128 matmuls;
    biases are added by fused DVE adds (no bias matmuls).
"""
import os
import math
import threading

import numpy as np
import ml_dtypes

import concourse.bass as bass
import concourse.tile as tile
from concourse import mybir, bacc, bass_utils

B, T, C, H = 2, 2048, 1024, 16
D = C // H                 # 64
NCORES = 8
HPC = H // NCORES          # heads per core = 2
HC = HPC * D               # head-channels per core = 128
BT = B * T                 # 4096
TQ = 512                   # query chunk
TKT = 128                  # key tile
ROWS = BT // NCORES        # output rows per core = 512
SM_SCALE = 1.0 / math.sqrt(D)
KT = C // 128              # 8 contraction tiles over channels
NCH = BT // TQ             # 8 T-chunks over B*T
SPC = TQ // D              # 8 strips of 64 rows per chunk (one per core)

F32 = mybir.dt.float32
BF16 = mybir.dt.bfloat16
BF16_NP = ml_dtypes.bfloat16


def _build_program():
    nc = bacc.Bacc("TRN2", target_bir_lowering=False, debug=False,
                   num_devices=NCORES)
    xt = nc.dram_tensor("xt", [C, BT], BF16, kind="ExternalInput").ap()
    wqkv = nc.dram_tensor("wqkv", [C, 3 * HC], BF16, kind="ExternalInput").ap()
    wproj = nc.dram_tensor("wproj", [C, C], BF16, kind="ExternalInput").ap()
    bqk = nc.dram_tensor("bqk", [HC, 2], F32, kind="ExternalInput").ap()
    bvb = nc.dram_tensor("bvb", [128, HC], BF16, kind="ExternalInput").ap()
    bpb = nc.dram_tensor("bpb", [128, C], BF16, kind="ExternalInput").ap()
    mask2 = nc.dram_tensor("mask2", [TKT, 2, TKT], BF16,
                           kind="ExternalInput").ap()
    outp = nc.dram_tensor("out", [ROWS, C], F32, kind="ExternalOutput").ap()

    with tile.TileContext(nc) as tc:
        with (
            tc.tile_pool(name="consts", bufs=1) as consts,
            tc.tile_pool(name="xpool", bufs=2) as xpool,
            tc.tile_pool(name="ppool", bufs=6) as ppool,
            tc.tile_pool(name="apool", bufs=3) as apool,
            tc.tile_pool(name="npool", bufs=2) as npool,
            tc.tile_pool(name="opool", bufs=2) as opool,
            tc.tile_pool(name="ps_ss", bufs=2, space="PSUM") as ps_ss,
            tc.tile_pool(name="ps_y", bufs=2, space="PSUM") as ps_y,
            tc.tile_pool(name="ps_sum", bufs=1, space="PSUM") as ps_sum,
            tc.tile_pool(name="ps_o", bufs=1, space="PSUM") as ps_o,
            tc.tile_pool(name="dram", bufs=1, space="DRAM") as dram,
        ):
            # ---- stage 0: weights & constants (single-trigger batched
            # DMAs: the ~600ns per-dma_start engine cost dominates startup
            # otherwise) ----
            bqk_sb = consts.tile([HC, 2], F32, name="bqk_sb")
            nc.sync.dma_start(out=bqk_sb, in_=bqk)
            bq_sb = bqk_sb[:, 0:1]
            bk_sb = bqk_sb[:, 1:2]
            # one medium DMA per section/kt rather than one huge transfer:
            # each trigger's packets land on one HW queue (~83 GB/s), so
            # several in-flight triggers parallelize across queues
            wqkv_sb = consts.tile([128, KT, 3 * HC], BF16, name="wqkv_sb")
            for sec in range(3):
                nc.sync.dma_start(
                    out=wqkv_sb[:, :, HC * sec:HC * (sec + 1)],
                    in_=wqkv[:, HC * sec:HC * (sec + 1)]
                        .rearrange("(k p) c -> p k c", p=128))
            # prefetch all xt chunk-pairs on the two hardware-DGE engines
            # (sync + scalar); gpsimd is software-DGE with slow, variable
            # descriptor generation — late-needed weights only
            xt_pair = {}

            def prefetch_xt(p):
                x1 = xpool.tile([128, KT, 2 * TQ], BF16, tag="xt",
                                name=f"xt_{p}")
                for kt in range(KT):
                    if p == 0:
                        eng = nc.sync if kt % 2 == 0 else nc.scalar
                    else:
                        eng = nc.sync if p == 2 else nc.scalar
                    eng.dma_start(
                        out=x1[:, kt, :],
                        in_=xt[128 * kt:128 * (kt + 1),
                               1024 * p:1024 * p + 1024])
                xt_pair[p] = x1

            # pairs 2/3 are prefetched mid-stage-1 instead, to spread the
            # HBM demand peak (all 8 cores pull ~13MB at startup)
            prefetch_xt(0)
            prefetch_xt(1)
            bvb_sb = consts.tile([128, HC], BF16, name="bvb_sb")
            nc.gpsimd.dma_start(out=bvb_sb, in_=bvb)
            # big weights not needed until late: after the xt prefetches
            wproj_sb = consts.tile([128, KT, C], BF16, name="wproj_sb")
            nc.gpsimd.dma_start(out=wproj_sb,
                                in_=wproj.rearrange("(k p) c -> p k c", p=128))
            bpb_sb = consts.tile([128, C], BF16, name="bpb_sb")
            nc.gpsimd.dma_start(out=bpb_sb, in_=bpb)
            mask_sb = consts.tile([TKT, 2, TKT], BF16, name="mask_sb")
            nc.gpsimd.dma_start(out=mask_sb, in_=mask2)
            ones64 = consts.tile([128, D], BF16, name="ones64")
            nc.vector.memset(ones64, 1.0)

            qT_b = [consts.tile([HC, T], BF16, name=f"qT_sb{b}")
                    for b in range(B)]
            kT_b = [consts.tile([HC, T], BF16, name=f"kT_sb{b}")
                    for b in range(B)]
            # v tiles: [128 keys, 128] = [v_h0 (64) | v_h1 (64)]
            v_sb = [consts.tile([128, HC], BF16, name=f"v_sb{tt}")
                    for tt in range(BT // 128)]

            # per-pair exchange buffers: block s holds both chunks' strips
            # (queries [64s, 64s+64) of each chunk) for core s
            a2a_in = [dram.tile([NCORES, 2, HC, D], BF16, name=f"a2a_in{p}")
                      for p in range(4)]
            a2a_out = [dram.tile([NCORES, 2, HC, D], BF16, name=f"a2a_out{p}")
                       for p in range(4)]

            # ---- stage 1: QKV projection (both batches up front) ----
            for b in range(B):
                for cp in range(2):                      # chunk pairs
                    if 2 * b + cp + 2 <= 3:
                        prefetch_xt(2 * b + cp + 2)
                    xx = xt_pair[2 * b + cp]
                    for half in range(2):
                        cl = 2 * cp + half               # chunk in batch
                        c = 4 * b + cl                   # global chunk
                        col = slice(TQ * half, TQ * (half + 1))
                        ps = ps_ss.tile([128, 2 * TQ], F32, tag="ss")
                        for kt in range(KT):
                            nc.tensor.matmul(
                                ps[:, 0:TQ],
                                lhsT=wqkv_sb[:, kt, 0:HC],
                                rhs=xx[:, kt, col],
                                start=(kt == 0), stop=(kt == KT - 1))
                        for kt in range(KT):
                            nc.tensor.matmul(
                                ps[:, TQ:2 * TQ],
                                lhsT=wqkv_sb[:, kt, HC:2 * HC],
                                rhs=xx[:, kt, col],
                                start=(kt == 0), stop=(kt == KT - 1))
                        nc.scalar.activation(
                            out=qT_b[b][:, TQ * cl:TQ * (cl + 1)],
                            in_=ps[:, 0:TQ],
                            func=mybir.ActivationFunctionType.Identity,
                            bias=bq_sb, scale=1.0)
                        nc.scalar.activation(
                            out=kT_b[b][:, TQ * cl:TQ * (cl + 1)],
                            in_=ps[:, TQ:2 * TQ],
                            func=mybir.ActivationFunctionType.Identity,
                            bias=bk_sb, scale=SM_SCALE)
                        for s in range(TQ // 128):
                            tt = 4 * c + s
                            pv = ps_sum.tile([128, HC], F32, tag="sum")
                            for kt in range(KT):
                                nc.tensor.matmul(
                                    pv,
                                    lhsT=xx[:, kt, TQ * half + 128 * s:
                                            TQ * half + 128 * (s + 1)],
                                    rhs=wqkv_sb[:, kt, 2 * HC:3 * HC],
                                    start=(kt == 0), stop=(kt == KT - 1))
                            nc.vector.tensor_add(v_sb[tt], pv, bvb_sb)

            # ---- stage 2: attention; two chunks in flight, biggest first;
            # each pair's exchange + output projection follows immediately
            # and hides under the next pair's attention ----
            def emit_S(b, jl, slot):
                """Issue both heads' score matmuls + exp (+ mask) for key
                tile `slot` of chunk jl; returns (pt tile, c0)."""
                i = slot
                q0 = TQ * jl
                k0 = TKT * i
                r = i - (TQ // TKT) * jl
                c0 = TKT * r if r >= 0 else 0
                ss = ps_ss.tile([128, 2 * TQ], F32, tag="ss")
                nc.tensor.matmul(
                    ss[:, c0:TQ],
                    lhsT=kT_b[b][0:D, k0:k0 + TKT],
                    rhs=qT_b[b][0:D, q0 + c0:q0 + TQ],
                    start=True, stop=True)
                nc.tensor.matmul(
                    ss[:, TQ + c0:2 * TQ],
                    lhsT=kT_b[b][D:2 * D, k0:k0 + TKT],
                    rhs=qT_b[b][D:2 * D, q0 + c0:q0 + TQ],
                    start=True, stop=True)
                pt = ppool.tile([128, 2, TQ], BF16, tag="pt")
                ssv = ss.rearrange("p (h q) -> p h q", h=2)
                nc.scalar.activation(
                    out=pt[:, :, c0:TQ], in_=ssv[:, :, c0:TQ],
                    func=mybir.ActivationFunctionType.Exp)
                if r >= 0:
                    nc.vector.tensor_mul(pt[:, :, c0:c0 + TKT],
                                         pt[:, :, c0:c0 + TKT], mask_sb)
                return pt, c0

            def emit_PV(b, jl, i, nkt, y_ps, acc, pt, c0):
                vt = v_sb[(T // 128) * b + i]
                ptf = pt.rearrange("p h q -> p (h q)")
                nc.tensor.matmul(
                    y_ps[0:D, c0:TQ],
                    lhsT=vt[:, 0:D],
                    rhs=ptf[:, c0:TQ],
                    start=(i == 0), stop=(i == nkt - 1),
                    skip_group_check=True)
                nc.tensor.matmul(
                    y_ps[D:2 * D, c0:TQ],
                    lhsT=vt[:, D:2 * D],
                    rhs=ptf[:, TQ + c0:2 * TQ],
                    start=(i == 0), stop=(i == nkt - 1),
                    skip_group_check=True)
                if i == 0:
                    nc.vector.tensor_copy(out=acc, in_=ptf)
                else:
                    accv = acc.rearrange("p (h q) -> p h q", h=2)
                    nc.vector.tensor_add(accv[:, :, c0:TQ],
                                         accv[:, :, c0:TQ],
                                         pt[:, :, c0:TQ])

            def finalize(pi, which, y_ps, acc):
                sums = ps_sum.tile([128, TQ], F32, tag="sum")
                nc.tensor.matmul(sums[0:D, :], lhsT=ones64,
                                 rhs=acc[:, 0:TQ], start=True, stop=True)
                nc.tensor.matmul(sums[D:2 * D, :], lhsT=ones64,
                                 rhs=acc[:, TQ:2 * TQ], start=True, stop=True)
                recip = npool.tile([128, TQ], F32, tag="recip")
                nc.vector.reciprocal_approx_fast(out=recip, in_=sums)
                yt = npool.tile([128, TQ], BF16, tag="yt")
                nc.vector.tensor_mul(yt, y_ps, recip)
                nc.gpsimd.dma_start(
                    out=a2a_in[pi][:, which].rearrange("s p q -> p s q"),
                    in_=yt.rearrange("p (s q) -> p s q", s=SPC))

            def stage4_pair(pi, cA, cB):
                """Output projection for two 64-row strips, M=128-packed."""
                yy = opool.tile([128, KT, 2 * D], BF16, tag="ylhs")
                nc.gpsimd.dma_start(
                    out=yy.rearrange("p k (w q) -> p k w q", w=2),
                    in_=a2a_out[pi].rearrange("k w p q -> p k w q"))
                osb = opool.tile([128, 2 * TQ], F32, tag="osb")
                for n in range(C // TQ):
                    po = ps_o.tile([128, TQ], F32, tag="po")
                    for kt in range(KT):
                        nc.tensor.matmul(
                            po,
                            lhsT=yy[:, kt, :],
                            rhs=wproj_sb[:, kt, TQ * n:TQ * (n + 1)],
                            start=(kt == 0), stop=(kt == KT - 1))
                    nc.vector.tensor_add(osb[:, TQ * n:TQ * (n + 1)], po,
                                         bpb_sb[:, TQ * n:TQ * (n + 1)])
                for ci, cc in enumerate((cA, cB)):
                    nc.sync.dma_start(
                        out=outp[D * cc:D * (cc + 1), :],
                        in_=osb[D * ci:D * (ci + 1), :])

            pending_s4 = []
            pi = -1
            for b in range(B):
                for jA, jB in ((1, 0), (3, 2)):
                    pi += 1
                    cA, cB = 4 * b + jA, 4 * b + jB
                    nktA, nktB = 4 * (jA + 1), 4 * (jB + 1)
                    yA = ps_y.tile([128, TQ], F32, tag="y", name="yA")
                    yB = ps_y.tile([128, TQ], F32, tag="y", name="yB")
                    accA = apool.tile([128, 2 * TQ], BF16, tag="acc",
                                      name="accA")
                    accB = apool.tile([128, 2 * TQ], BF16, tag="acc",
                                      name="accB")
                    prevA = prevB = None
                    for i in range(nktA):
                        curA = emit_S(b, jA, i)
                        if prevA is not None:
                            emit_PV(b, jA, i - 1, nktA, yA, accA, *prevA)
                        prevA = curA
                        if i < nktB:
                            curB = emit_S(b, jB, i)
                            if prevB is not None:
                                emit_PV(b, jB, i - 1, nktB, yB, accB, *prevB)
                            prevB = curB
                        elif i == nktB:
                            # B is done: flush its last PV and stage its half
                            # of the pair's exchange buffer immediately
                            emit_PV(b, jB, nktB - 1, nktB, yB, accB, *prevB)
                            finalize(pi, 1, yB, accB)
                        if pi == 3 and i in (10, 14) and pending_s4:
                            # earlier pairs' output projections fire late in
                            # the LAST pair's attention: their a2a exchanges
                            # (even the skew-absorbing first one) are done by
                            # now, so the matmuls never head-of-line block
                            # the PE queue, and pairs 0-2 run stall-free
                            pending_s4.pop(0)()
                    emit_PV(b, jA, nktA - 1, nktA, yA, accA, *prevA)
                    finalize(pi, 0, yA, accA)
                    nc.gpsimd.collective_compute(
                        "AllToAll", mybir.AluOpType.bypass,
                        replica_groups=[list(range(NCORES))],
                        ins=[a2a_in[pi].opt()],
                        outs=[a2a_out[pi].opt()])
                    pending_s4.append(lambda pi=pi, cA=cA, cB=cB:
                                      stage4_pair(pi, cA, cB))
            for s4 in pending_s4:
                s4()

    nc.compile()
    return nc


_lock = threading.Lock()
_cached_nc = None
last_results = None  # BassKernelResults of the most recent kernel() call


def _get_program():
    global _cached_nc
    with _lock:
        if _cached_nc is None:
            _cached_nc = _build_program()
    return _cached_nc


def _host_inputs(x, W_qkv, b_qkv, W_proj, b_proj):
    bf = lambda a: np.ascontiguousarray(a).astype(BF16_NP)
    x = np.asarray(x, dtype=np.float32)
    W_qkv = np.asarray(W_qkv, dtype=np.float32)
    b_qkv = np.asarray(b_qkv, dtype=np.float32)
    W_proj = np.asarray(W_proj, dtype=np.float32)
    b_proj = np.asarray(b_proj, dtype=np.float32)

    xt = bf(x.reshape(BT, C).T)                     # [C, BT]
    wproj = bf(W_proj)                              # [C, C]
    bpb = bf(np.broadcast_to(b_proj.reshape(1, C), (128, C)))
    k = np.arange(TKT)[:, None, None]
    q = np.arange(TKT)[None, None, :]
    mask2 = np.broadcast_to(k <= q, (TKT, 2, TKT)).astype(BF16_NP)

    in_maps = []
    for i in range(NCORES):
        sel = slice(HC * i, HC * (i + 1))
        wq = W_qkv[:, sel]
        wk = W_qkv[:, C + HC * i:C + HC * (i + 1)]
        wv = W_qkv[:, 2 * C + HC * i:2 * C + HC * (i + 1)]
        bv = b_qkv[2 * C + HC * i:2 * C + HC * (i + 1)]
        in_maps.append({
            "xt": xt,
            "wqkv": bf(np.concatenate([wq, wk, wv], axis=1)),
            "wproj": wproj,
            "bqk": np.ascontiguousarray(np.stack(
                [b_qkv[sel],
                 b_qkv[C + HC * i:C + HC * (i + 1)] * SM_SCALE],
                axis=1)).astype(np.float32),
            "bvb": bf(np.broadcast_to(bv.reshape(1, HC), (128, HC))),
            "bpb": bpb,
            "mask2": np.ascontiguousarray(mask2),
        })
    return in_maps


def kernel(x, W_qkv, b_qkv, W_proj, b_proj):
    global last_results
    nc = _get_program()
    in_maps = _host_inputs(x, W_qkv, b_qkv, W_proj, b_proj)
    trace = bool(int(os.environ.get("KERNEL_TRACE", "0")))
    res = bass_utils.run_bass_kernel_spmd(
        nc, in_maps, core_ids=list(range(NCORES)), trace=trace)
    last_results = res
    # core s's output rows are strip s (64 rows) of every 512-row chunk
    arr = np.stack([res.results[s]["out"].reshape(BT // TQ, D, C)
                    for s in range(NCORES)], axis=1)   # [chunk, core, 64, C]
    return np.ascontiguousarray(arr.reshape(B, T, C))


# revision 28
# speedup vs baseline: 1.2473x; 1.0147x over previous
"""Causal self-attention (B=2, T=2048, C=1024, H=16) on 8 TRN2 NeuronCores.

Sharding: tensor-parallel over heads (2 heads/core) for QKV projection and
attention; AllToAll converts the head-sharded attention output into a
sequence-sharded layout; each core then computes its 512-row slice of the
output projection. Host only slices/casts inputs and concatenates outputs.

Device math in bf16 with fp32 PSUM accumulation:
  - x is pre-transposed on host to xT [C, B*T] (bf16) so every matmul
    contraction has channels on the partition axis.
  - Scores are built transposed (S^T [keys, queries]); both heads' score
    matmuls are row-tiled (K=64 halves of the PE array) and run
    concurrently; both land in one 2-bank PSUM tile so a single exp
    covers both heads.
  - PV matmuls are col-tiled (M=64 halves) and run concurrently.
  - Softmax sums: P tiles are accumulated on DVE; one col-tiled
    broadcast-matmul pair per chunk produces [128,512] broadcast sums,
    so normalization is two full-width DVE ops (reciprocal + mul).
  - Causal slicing: diagonal key-tiles only compute/exp the valid
    query-column range; a single [128,2,128] tril slab handles masking.
  - Output projection packs two 64-row query strips into M=128 matmuls;
    biases are added by fused DVE adds (no bias matmuls).
"""
import os
import math
import threading

import numpy as np
import ml_dtypes

import concourse.bass as bass
import concourse.tile as tile
from concourse import mybir, bacc, bass_utils

B, T, C, H = 2, 2048, 1024, 16
D = C // H                 # 64
NCORES = 8
HPC = H // NCORES          # heads per core = 2
HC = HPC * D               # head-channels per core = 128
BT = B * T                 # 4096
TQ = 512                   # query chunk
TKT = 128                  # key tile
ROWS = BT // NCORES        # output rows per core = 512
SM_SCALE = 1.0 / math.sqrt(D)
KT = C // 128              # 8 contraction tiles over channels
NCH = BT // TQ             # 8 T-chunks over B*T
SPC = TQ // D              # 8 strips of 64 rows per chunk (one per core)

F32 = mybir.dt.float32
BF16 = mybir.dt.bfloat16
BF16_NP = ml_dtypes.bfloat16


def _build_program():
    nc = bacc.Bacc("TRN2", target_bir_lowering=False, debug=False,
                   num_devices=NCORES)
    xt = nc.dram_tensor("xt", [C, BT], BF16, kind="ExternalInput").ap()
    wqkv = nc.dram_tensor("wqkv", [C, 3 * HC], BF16, kind="ExternalInput").ap()
    wproj = nc.dram_tensor("wproj", [C, C], BF16, kind="ExternalInput").ap()
    bqk = nc.dram_tensor("bqk", [HC, 2], F32, kind="ExternalInput").ap()
    bvb = nc.dram_tensor("bvb", [128, HC], BF16, kind="ExternalInput").ap()
    bpb = nc.dram_tensor("bpb", [128, C], BF16, kind="ExternalInput").ap()
    mask2 = nc.dram_tensor("mask2", [TKT, 2, TKT], BF16,
                           kind="ExternalInput").ap()
    outp = nc.dram_tensor("out", [ROWS, C], F32, kind="ExternalOutput").ap()

    with tile.TileContext(nc) as tc:
        with (
            tc.tile_pool(name="consts", bufs=1) as consts,
            tc.tile_pool(name="xpool", bufs=2) as xpool,
            tc.tile_pool(name="ppool", bufs=6) as ppool,
            tc.tile_pool(name="apool", bufs=3) as apool,
            tc.tile_pool(name="npool", bufs=4) as npool,
            tc.tile_pool(name="opool", bufs=2) as opool,
            tc.tile_pool(name="ps_ss", bufs=2, space="PSUM") as ps_ss,
            tc.tile_pool(name="ps_y", bufs=2, space="PSUM") as ps_y,
            tc.tile_pool(name="ps_sum", bufs=1, space="PSUM") as ps_sum,
            tc.tile_pool(name="ps_o", bufs=1, space="PSUM") as ps_o,
            tc.tile_pool(name="dram", bufs=1, space="DRAM") as dram,
        ):
            # ---- stage 0: weights & constants (single-trigger batched
            # DMAs: the ~600ns per-dma_start engine cost dominates startup
            # otherwise) ----
            bqk_sb = consts.tile([HC, 2], F32, name="bqk_sb")
            nc.sync.dma_start(out=bqk_sb, in_=bqk)
            bq_sb = bqk_sb[:, 0:1]
            bk_sb = bqk_sb[:, 1:2]
            # one medium DMA per section/kt rather than one huge transfer:
            # each trigger's packets land on one HW queue (~83 GB/s), so
            # several in-flight triggers parallelize across queues
            wqkv_sb = consts.tile([128, KT, 3 * HC], BF16, name="wqkv_sb")
            for sec in range(3):
                nc.sync.dma_start(
                    out=wqkv_sb[:, :, HC * sec:HC * (sec + 1)],
                    in_=wqkv[:, HC * sec:HC * (sec + 1)]
                        .rearrange("(k p) c -> p k c", p=128))
            # prefetch all xt chunk-pairs on the two hardware-DGE engines
            # (sync + scalar); gpsimd is software-DGE with slow, variable
            # descriptor generation — late-needed weights only
            xt_pair = {}

            def prefetch_xt(p):
                x1 = xpool.tile([128, KT, 2 * TQ], BF16, tag="xt",
                                name=f"xt_{p}")
                for kt in range(KT):
                    if p == 0:
                        eng = nc.sync if kt % 2 == 0 else nc.scalar
                    else:
                        eng = nc.sync if p == 2 else nc.scalar
                    eng.dma_start(
                        out=x1[:, kt, :],
                        in_=xt[128 * kt:128 * (kt + 1),
                               1024 * p:1024 * p + 1024])
                xt_pair[p] = x1

            # pairs 2/3 are prefetched mid-stage-1 instead, to spread the
            # HBM demand peak (all 8 cores pull ~13MB at startup)
            prefetch_xt(0)
            prefetch_xt(1)
            bvb_sb = consts.tile([128, HC], BF16, name="bvb_sb")
            nc.gpsimd.dma_start(out=bvb_sb, in_=bvb)
            # big weights not needed until late: after the xt prefetches
            wproj_sb = consts.tile([128, KT, C], BF16, name="wproj_sb")
            nc.gpsimd.dma_start(out=wproj_sb,
                                in_=wproj.rearrange("(k p) c -> p k c", p=128))
            bpb_sb = consts.tile([128, C], BF16, name="bpb_sb")
            nc.gpsimd.dma_start(out=bpb_sb, in_=bpb)
            mask_sb = consts.tile([TKT, 2, TKT], BF16, name="mask_sb")
            nc.gpsimd.dma_start(out=mask_sb, in_=mask2)
            ones64 = consts.tile([128, D], BF16, name="ones64")
            nc.vector.memset(ones64, 1.0)

            qT_b = [consts.tile([HC, T], BF16, name=f"qT_sb{b}")
                    for b in range(B)]
            kT_b = [consts.tile([HC, T], BF16, name=f"kT_sb{b}")
                    for b in range(B)]
            # v tiles: [128 keys, 128] = [v_h0 (64) | v_h1 (64)]
            v_sb = [consts.tile([128, HC], BF16, name=f"v_sb{tt}")
                    for tt in range(BT // 128)]

            # per-pair exchange buffers: block s holds both chunks' strips
            # (queries [64s, 64s+64) of each chunk) for core s
            a2a_in = [dram.tile([NCORES, 2, HC, D], BF16, name=f"a2a_in{p}")
                      for p in range(4)]
            a2a_out = [dram.tile([NCORES, 2, HC, D], BF16, name=f"a2a_out{p}")
                       for p in range(4)]

            # ---- stage 1: QKV projection (both batches up front) ----
            for b in range(B):
                for cp in range(2):                      # chunk pairs
                    if 2 * b + cp + 2 <= 3:
                        prefetch_xt(2 * b + cp + 2)
                    xx = xt_pair[2 * b + cp]
                    for half in range(2):
                        cl = 2 * cp + half               # chunk in batch
                        c = 4 * b + cl                   # global chunk
                        col = slice(TQ * half, TQ * (half + 1))
                        ps = ps_ss.tile([128, 2 * TQ], F32, tag="ss")
                        for kt in range(KT):
                            nc.tensor.matmul(
                                ps[:, 0:TQ],
                                lhsT=wqkv_sb[:, kt, 0:HC],
                                rhs=xx[:, kt, col],
                                start=(kt == 0), stop=(kt == KT - 1))
                        for kt in range(KT):
                            nc.tensor.matmul(
                                ps[:, TQ:2 * TQ],
                                lhsT=wqkv_sb[:, kt, HC:2 * HC],
                                rhs=xx[:, kt, col],
                                start=(kt == 0), stop=(kt == KT - 1))
                        nc.scalar.activation(
                            out=qT_b[b][:, TQ * cl:TQ * (cl + 1)],
                            in_=ps[:, 0:TQ],
                            func=mybir.ActivationFunctionType.Identity,
                            bias=bq_sb, scale=1.0)
                        nc.scalar.activation(
                            out=kT_b[b][:, TQ * cl:TQ * (cl + 1)],
                            in_=ps[:, TQ:2 * TQ],
                            func=mybir.ActivationFunctionType.Identity,
                            bias=bk_sb, scale=SM_SCALE)
                        for s in range(TQ // 128):
                            tt = 4 * c + s
                            pv = ps_sum.tile([128, HC], F32, tag="sum")
                            for kt in range(KT):
                                nc.tensor.matmul(
                                    pv,
                                    lhsT=xx[:, kt, TQ * half + 128 * s:
                                            TQ * half + 128 * (s + 1)],
                                    rhs=wqkv_sb[:, kt, 2 * HC:3 * HC],
                                    start=(kt == 0), stop=(kt == KT - 1))
                            nc.vector.tensor_add(v_sb[tt], pv, bvb_sb)

            # ---- stage 2: attention; two chunks in flight, biggest first;
            # each pair's exchange + output projection follows immediately
            # and hides under the next pair's attention ----
            def emit_S(b, jl, slot):
                """Issue both heads' score matmuls + exp (+ mask) for key
                tile `slot` of chunk jl; returns (pt tile, c0)."""
                i = slot
                q0 = TQ * jl
                k0 = TKT * i
                r = i - (TQ // TKT) * jl
                c0 = TKT * r if r >= 0 else 0
                ss = ps_ss.tile([128, 2 * TQ], F32, tag="ss")
                nc.tensor.matmul(
                    ss[:, c0:TQ],
                    lhsT=kT_b[b][0:D, k0:k0 + TKT],
                    rhs=qT_b[b][0:D, q0 + c0:q0 + TQ],
                    start=True, stop=True)
                nc.tensor.matmul(
                    ss[:, TQ + c0:2 * TQ],
                    lhsT=kT_b[b][D:2 * D, k0:k0 + TKT],
                    rhs=qT_b[b][D:2 * D, q0 + c0:q0 + TQ],
                    start=True, stop=True)
                pt = ppool.tile([128, 2, TQ], BF16, tag="pt")
                ssv = ss.rearrange("p (h q) -> p h q", h=2)
                nc.scalar.activation(
                    out=pt[:, :, c0:TQ], in_=ssv[:, :, c0:TQ],
                    func=mybir.ActivationFunctionType.Exp)
                if r >= 0:
                    nc.vector.tensor_mul(pt[:, :, c0:c0 + TKT],
                                         pt[:, :, c0:c0 + TKT], mask_sb)
                return pt, c0

            def emit_PV(b, jl, i, nkt, y_ps, acc, pt, c0):
                vt = v_sb[(T // 128) * b + i]
                ptf = pt.rearrange("p h q -> p (h q)")
                nc.tensor.matmul(
                    y_ps[0:D, c0:TQ],
                    lhsT=vt[:, 0:D],
                    rhs=ptf[:, c0:TQ],
                    start=(i == 0), stop=(i == nkt - 1),
                    skip_group_check=True)
                nc.tensor.matmul(
                    y_ps[D:2 * D, c0:TQ],
                    lhsT=vt[:, D:2 * D],
                    rhs=ptf[:, TQ + c0:2 * TQ],
                    start=(i == 0), stop=(i == nkt - 1),
                    skip_group_check=True)
                if i == 0:
                    nc.vector.tensor_copy(out=acc, in_=ptf)
                else:
                    accv = acc.rearrange("p (h q) -> p h q", h=2)
                    nc.vector.tensor_add(accv[:, :, c0:TQ],
                                         accv[:, :, c0:TQ],
                                         pt[:, :, c0:TQ])

            def finalize(pi, which, y_ps, acc):
                sums = ps_sum.tile([128, TQ], F32, tag="sum")
                nc.tensor.matmul(sums[0:D, :], lhsT=ones64,
                                 rhs=acc[:, 0:TQ], start=True, stop=True)
                nc.tensor.matmul(sums[D:2 * D, :], lhsT=ones64,
                                 rhs=acc[:, TQ:2 * TQ], start=True, stop=True)
                recip = npool.tile([128, TQ], F32, tag="recip")
                nc.vector.reciprocal_approx_fast(out=recip, in_=sums)
                yt = npool.tile([128, TQ], BF16, tag="yt")
                nc.vector.tensor_mul(yt, y_ps, recip)
                # staging write rides the sync HWDGE queue (idle during
                # attention; the gpsimd SWDGE path is slow for the 128B
                # packets this strided layout produces)
                nc.sync.dma_start(
                    out=a2a_in[pi][:, which].rearrange("s p q -> p s q"),
                    in_=yt.rearrange("p (s q) -> p s q", s=SPC))

            def stage4_pair(pi, cA, cB):
                """Output projection for two 64-row strips, M=128-packed."""
                yy = opool.tile([128, KT, 2 * D], BF16, tag="ylhs")
                nc.gpsimd.dma_start(
                    out=yy.rearrange("p k (w q) -> p k w q", w=2),
                    in_=a2a_out[pi].rearrange("k w p q -> p k w q"))
                osb = opool.tile([128, 2 * TQ], F32, tag="osb")
                for n in range(C // TQ):
                    po = ps_o.tile([128, TQ], F32, tag="po")
                    for kt in range(KT):
                        nc.tensor.matmul(
                            po,
                            lhsT=yy[:, kt, :],
                            rhs=wproj_sb[:, kt, TQ * n:TQ * (n + 1)],
                            start=(kt == 0), stop=(kt == KT - 1))
                    nc.vector.tensor_add(osb[:, TQ * n:TQ * (n + 1)], po,
                                         bpb_sb[:, TQ * n:TQ * (n + 1)])
                for ci, cc in enumerate((cA, cB)):
                    nc.sync.dma_start(
                        out=outp[D * cc:D * (cc + 1), :],
                        in_=osb[D * ci:D * (ci + 1), :])

            pending_s4 = []
            pi = -1
            for b in range(B):
                for jA, jB in ((1, 0), (3, 2)):
                    pi += 1
                    cA, cB = 4 * b + jA, 4 * b + jB
                    nktA, nktB = 4 * (jA + 1), 4 * (jB + 1)
                    yA = ps_y.tile([128, TQ], F32, tag="y", name="yA")
                    yB = ps_y.tile([128, TQ], F32, tag="y", name="yB")
                    accA = apool.tile([128, 2 * TQ], BF16, tag="acc",
                                      name="accA")
                    accB = apool.tile([128, 2 * TQ], BF16, tag="acc",
                                      name="accB")
                    prevA = prevB = None
                    for i in range(nktA):
                        curA = emit_S(b, jA, i)
                        if prevA is not None:
                            emit_PV(b, jA, i - 1, nktA, yA, accA, *prevA)
                        prevA = curA
                        if i < nktB:
                            curB = emit_S(b, jB, i)
                            if prevB is not None:
                                emit_PV(b, jB, i - 1, nktB, yB, accB, *prevB)
                            prevB = curB
                        elif i == nktB:
                            # B is done: flush its last PV and stage its half
                            # of the pair's exchange buffer immediately
                            emit_PV(b, jB, nktB - 1, nktB, yB, accB, *prevB)
                            finalize(pi, 1, yB, accB)
                        if pi == 3 and i in (10, 14) and pending_s4:
                            # earlier pairs' output projections fire late in
                            # the LAST pair's attention: their a2a exchanges
                            # (even the skew-absorbing first one) are done by
                            # now, so the matmuls never head-of-line block
                            # the PE queue, and pairs 0-2 run stall-free
                            pending_s4.pop(0)()
                    emit_PV(b, jA, nktA - 1, nktA, yA, accA, *prevA)
                    finalize(pi, 0, yA, accA)
                    nc.gpsimd.collective_compute(
                        "AllToAll", mybir.AluOpType.bypass,
                        replica_groups=[list(range(NCORES))],
                        ins=[a2a_in[pi].opt()],
                        outs=[a2a_out[pi].opt()])
                    pending_s4.append(lambda pi=pi, cA=cA, cB=cB:
                                      stage4_pair(pi, cA, cB))
            for s4 in pending_s4:
                s4()

    nc.compile()
    return nc


_lock = threading.Lock()
_cached_nc = None
last_results = None  # BassKernelResults of the most recent kernel() call


def _get_program():
    global _cached_nc
    with _lock:
        if _cached_nc is None:
            _cached_nc = _build_program()
    return _cached_nc


def _host_inputs(x, W_qkv, b_qkv, W_proj, b_proj):
    bf = lambda a: np.ascontiguousarray(a).astype(BF16_NP)
    x = np.asarray(x, dtype=np.float32)
    W_qkv = np.asarray(W_qkv, dtype=np.float32)
    b_qkv = np.asarray(b_qkv, dtype=np.float32)
    W_proj = np.asarray(W_proj, dtype=np.float32)
    b_proj = np.asarray(b_proj, dtype=np.float32)

    xt = bf(x.reshape(BT, C).T)                     # [C, BT]
    wproj = bf(W_proj)                              # [C, C]
    bpb = bf(np.broadcast_to(b_proj.reshape(1, C), (128, C)))
    k = np.arange(TKT)[:, None, None]
    q = np.arange(TKT)[None, None, :]
    mask2 = np.broadcast_to(k <= q, (TKT, 2, TKT)).astype(BF16_NP)

    in_maps = []
    for i in range(NCORES):
        sel = slice(HC * i, HC * (i + 1))
        wq = W_qkv[:, sel]
        wk = W_qkv[:, C + HC * i:C + HC * (i + 1)]
        wv = W_qkv[:, 2 * C + HC * i:2 * C + HC * (i + 1)]
        bv = b_qkv[2 * C + HC * i:2 * C + HC * (i + 1)]
        in_maps.append({
            "xt": xt,
            "wqkv": bf(np.concatenate([wq, wk, wv], axis=1)),
            "wproj": wproj,
            "bqk": np.ascontiguousarray(np.stack(
                [b_qkv[sel],
                 b_qkv[C + HC * i:C + HC * (i + 1)] * SM_SCALE],
                axis=1)).astype(np.float32),
            "bvb": bf(np.broadcast_to(bv.reshape(1, HC), (128, HC))),
            "bpb": bpb,
            "mask2": np.ascontiguousarray(mask2),
        })
    return in_maps


def kernel(x, W_qkv, b_qkv, W_proj, b_proj):
    global last_results
    nc = _get_program()
    in_maps = _host_inputs(x, W_qkv, b_qkv, W_proj, b_proj)
    trace = bool(int(os.environ.get("KERNEL_TRACE", "0")))
    res = bass_utils.run_bass_kernel_spmd(
        nc, in_maps, core_ids=list(range(NCORES)), trace=trace)
    last_results = res
    # core s's output rows are strip s (64 rows) of every 512-row chunk
    arr = np.stack([res.results[s]["out"].reshape(BT // TQ, D, C)
                    for s in range(NCORES)], axis=1)   # [chunk, core, 64, C]
    return np.ascontiguousarray(arr.reshape(B, T, C))
